# revision 67
# baseline (speedup 1.0000x reference)
"""Causal GQA attention (B=2, S=2048, 32 q-heads, 8 kv-heads, D=128) on 8 TRN2 cores.

Sharding: tensor-parallel over kv heads - core i gets kv head i plus its 4
query heads (q cols [512i, 512i+512), k/v cols [128i, 128i+128)). Each core
computes its heads' attention independently; outputs concatenate on axis 1.

Per-core kernel, ~141.5us on the CoreSim cost model (baseline 159.1us).
Structure:
  - QK stays bf16 (K^T/Q^T via PE transposes, 8-tile groups through one
    PSUM bank, bf16 drains on DVE at 2x).
  - P is split by precision need: off-diagonal strips are fp8e4m3 and feed
    fp8 DoubleRow PV matmuls over k-tile PAIRS (0.5 cyc/row, ~4x fewer PE
    cycles); diagonal strips (and V for them) are bf16 singles, which keeps
    the short early rows accurate (fp8 V error is unaveraged at q~0).
  - exp runs on two engines concurrently: ACT does off-diagonal strips with
    the exact table exp (fp8 out, exp(SCALE*s - 5*ln2); the 2^-5 cancels in
    the softmax ratio and keeps max P inside fp8e4m3's 240). DVE does the
    diagonal strips with a 2-instruction Schraudolph exp: pass1 writes
    int32(s*A + Bmask) - the float->int convert-on-write builds the exp bit
    pattern, and causal masking is free because scalar_tensor_tensor adds a
    per-element B-mask whose invalid entries underflow the result to 0;
    pass2 (on Pool/gpsimd) copies the int32 tile bitcast-as-f32 out to bf16.
  - per 512-q block: off-diag strips pack into 1024-wide (2-bank) score
    tiles, the diagonal suffixes into 512+768 units ([384|128|256] stays
    bank-contained); three 2-bank score regions rotate so QK runs one unit
    ahead of exp (software pipeline), with PV pieces pumped between units.
  - PV output slots live in one PSUM bank (3x[128,132], ones-column gives
    the softmax denominator); reciprocals are batched one per slot-cycle
    (the per-instruction PSUM-access charge otherwise dominates), the
    normalizing multiply drains PSUM->SBUF on DVE, and each block's 512
    output rows ship in a single DMA on the SP ring (loads share it).
"""

import os
import sys

sys.path.insert(0, "/opt/trn_rl_repo")

KSKIP = set(os.environ.get("KSKIP", "").split(","))

from contextlib import ExitStack

import numpy as np

import concourse.bass as bass
import concourse.mybir as mybir
from concourse import bacc
import concourse.tile as tile
from concourse.bass_utils import run_bass_kernel_spmd
from concourse.masks import make_identity

F32 = mybir.dt.float32
BF16 = mybir.dt.bfloat16
FP8 = mybir.dt.float8e4
I32 = mybir.dt.int32

NUM_HEADS = 32
HEAD_DIM = 128
NUM_KV_HEADS = 8
SCALE = 0.08838834764831845  # 1/sqrt(128)
SEQ = 2048
TOK = 4096
B = TOK // SEQ  # 2 sequences
N_CORES = 8
G = NUM_HEADS // NUM_KV_HEADS  # 4 query heads per kv head (= per core)
SQ = SEQ // 128  # 16 128-token tiles per sequence
NQB = SEQ // 512  # 4 512-wide q blocks per sequence
EXP = mybir.ActivationFunctionType.Exp
MUL = mybir.AluOpType.mult
ADD = mybir.AluOpType.add
DR = mybir.MatmulPerfMode.DoubleRow

# Schraudolph exp constants: int32(s*A + B) bitcast to f32 ~= exp(SCALE*s).
# P carries a global 2^-3 factor (numerator and denominator both scale, the
# softmax ratio is invariant) so the largest exp stays well inside fp8e4m3.
LOG2E = 1.4426950408889634
SIGMA = 0.0434609
PSHIFT = 5.0  # fp8e4 (IEEE e4m3) max normal is 240; keep max P ~tens
A_CONST = float(SCALE * LOG2E * (1 << 23))
B_CONST = float((127.0 - SIGMA - PSHIFT) * (1 << 23))
B_LOW = float(B_CONST - 50.0 * (1 << 23))  # masked: underflows to 0 in fp8
ACT_BIAS = float(-PSHIFT * 0.6931471805599453)  # exp(SCALE*s - 3*ln2)

# diagonal pack layout (within the per-j diag PSUM tile / P region):
# gap-free order [m0:512 | m1:384 | m3:128 | m2:256] keeps every strip
# inside one 512-f32 PSUM bank
DIAG_OFF = (0, 512, 1024, 896)
DIAG_W = (512, 384, 256, 128)
DIAG_PACK_W = 1280

P_W = 12 * 512 + DIAG_PACK_W  # widest per-j P region (j=3)




def off_tiling(j):
    """Off-diagonal strip grouping into PSUM tiles: (first_tile, n_strips).
    Uniform 2-strip (1024 = 2 PSUM banks) tiles so three score regions fit
    alongside the PV-output and transpose banks."""
    return [(t, 2) for t in range(0, 4 * j, 2)]


def _body(ctx, tc, q, k, v, bmask, out):
    nc = tc.nc
    const = ctx.enter_context(tc.tile_pool(name="const", bufs=1))
    stage = ctx.enter_context(tc.tile_pool(name="stage", bufs=4))
    stagebf = ctx.enter_context(tc.tile_pool(name="stagebf", bufs=3))
    ktr_pool = ctx.enter_context(tc.tile_pool(name="ktr", bufs=2))
    qtr_pool = ctx.enter_context(tc.tile_pool(name="qtr", bufs=2))
    vaug_pool = ctx.enter_context(tc.tile_pool(name="vaug", bufs=4))
    pt_pool = ctx.enter_context(tc.tile_pool(name="pt", bufs=4))
    ptd_pool = ctx.enter_context(tc.tile_pool(name="ptd", bufs=4))
    stag_pool = ctx.enter_context(tc.tile_pool(name="stag", bufs=4))
    outsb_pool = ctx.enter_context(tc.tile_pool(name="outsb", bufs=8))
    rc_pool = ctx.enter_context(tc.tile_pool(name="rc", bufs=12))
    # PSUM: three 2-bank score regions (3-deep pipeline), 1 bank of 3
    # PV-output slots, 1 transpose staging bank.
    psum_a = ctx.enter_context(tc.tile_pool(name="psum_a", bufs=1, space="PSUM"))
    psum_b = ctx.enter_context(tc.tile_pool(name="psum_b", bufs=1, space="PSUM"))
    psum_c = ctx.enter_context(tc.tile_pool(name="psum_c", bufs=1, space="PSUM"))
    psum_o = ctx.enter_context(tc.tile_pool(name="psum_o", bufs=1, space="PSUM"))
    psum_t = ctx.enter_context(tc.tile_pool(name="psum_t", bufs=1, space="PSUM"))

    ident = const.tile([128, 128], BF16)
    make_identity(nc, ident)
    bm = const.tile([128, DIAG_PACK_W], F32)
    nc.sync.dma_start(out=bm, in_=bmask[:, :])
    bias_ap = const.tile([128, 1], F32, name="actbias")
    nc.vector.memset(bias_ap, ACT_BIAS)
    po_banks = [psum_o.tile([128, 3, 132], F32, name="po_bank0", bufs=1)]

    ab_state = [0]  # score-region rotation

    def score_tile(width):
        pool, tag = (
            (psum_a, "psa"),
            (psum_b, "psb"),
            (psum_c, "psc"),
        )[ab_state[0] % 3]
        ab_state[0] += 1
        return pool.tile([128, 1024], F32, tag=tag, name=tag)

    def build_tr(src_bf, dst_tr, grp8, drain=None):
        """Transpose 8 [128,128] bf16 tiles via one full PSUM bank, one
        drain. Few, widely-spaced builds keep the PSUM-bank WAR (which
        head-of-line blocks the in-order PE queue) off the critical path."""
        pst = psum_t.tile([128, 1024], BF16, tag="pst")
        for i in range(8):
            nc.tensor.transpose(
                out=pst[:, i * 128 : (i + 1) * 128],
                in_=src_bf[:, grp8 * 8 + i, :],
                identity=ident,
            )
        eng = drain or nc.vector  # Pool cannot read PSUM
        eng.tensor_copy(out=dst_tr[:, grp8 * 1024 : (grp8 + 1) * 1024], in_=pst)

    def load_cast(src_rows_ap, split=False):
        st = stage.tile([128, SQ, 128], F32, tag="stage")
        bf = stagebf.tile([128, SQ, 128], BF16, tag="stagebf")
        tiled = src_rows_ap.rearrange("(t p) d -> p t d", p=128)
        chunks = [(0, 4), (4, SQ)] if split else [(0, SQ)]
        for t0, t1 in chunks:
            nc.sync.dma_start(out=st[:, t0:t1, :], in_=tiled[:, t0:t1, :])
            nc.gpsimd.tensor_copy(out=bf[:, t0:t1, :], in_=st[:, t0:t1, :])
        return bf

    def seq_loads(b, dst, split=False):
        """DMA+cast K, first-head Q, and V(aug fp8) for sequence b into dst."""
        rows = slice(b * SEQ, (b + 1) * SEQ)
        if split:
            kst = stage.tile([128, SQ, 128], F32, tag="stage", name="kst")
            kbf = stagebf.tile([128, SQ, 128], BF16, tag="stagebf", name="kbf")
            qst = stage.tile([128, SQ, 128], F32, tag="stage", name="qst")
            qbf = stagebf.tile([128, SQ, 128], BF16, tag="stagebf", name="qbf")
            ktl = k[rows, :].rearrange("(t p) d -> p t d", p=128)
            qtl = q[rows, 0:128].rearrange("(t p) d -> p t d", p=128)
            for t0, t1 in ((0, 4), (4, 8), (8, 12), (12, SQ)):
                nc.sync.dma_start(out=kst[:, t0:t1, :], in_=ktl[:, t0:t1, :])
                nc.sync.dma_start(out=qst[:, t0:t1, :], in_=qtl[:, t0:t1, :])
                nc.gpsimd.tensor_copy(out=kbf[:, t0:t1, :], in_=kst[:, t0:t1, :])
                nc.gpsimd.tensor_copy(out=qbf[:, t0:t1, :], in_=qst[:, t0:t1, :])
            dst["kbf"], dst["qbf0"] = kbf, qbf
        else:
            dst["kbf"] = load_cast(k[rows, :])
            dst["qbf0"] = load_cast(q[rows, 0:128])
        vst = stage.tile([128, SQ, 128], F32, tag="stage")
        nc.sync.dma_start(out=vst, in_=v[rows, :].rearrange("(t p) d -> p t d", p=128))
        vaug = vaug_pool.tile([128, SQ, 132], FP8, name="vaug8")
        nc.gpsimd.tensor_copy(out=vaug[:, :, 0:128], in_=vst)
        nc.gpsimd.memset(vaug[:, :, 128:129], 1.0)
        dst["vaug"] = vaug
        # bf16 copy for the diagonal PV singles (fp8 V error is unaveraged on
        # the first rows of each block; bf16 V + bf16 P fixes them)
        vaugb = vaug_pool.tile([128, SQ, 132], BF16, name="vaugb")
        nc.gpsimd.tensor_copy(out=vaugb[:, :, 0:128], in_=vst)
        nc.gpsimd.memset(vaugb[:, :, 128:129], 1.0)
        dst["vaugb"] = vaugb

    def seq_alloc(dst):
        dst["ktr"] = ktr_pool.tile([128, SQ * 128], BF16, tag="ktr", name="ktr")
        dst["qtr0"] = qtr_pool.tile([128, SQ * 128], BF16, tag="qtr", name="qtr0")

    def seq_transposes(dst, grps):
        for grp in grps:
            build_tr(dst["kbf"], dst["ktr"], grp)
            build_tr(dst["qbf0"], dst["qtr0"], grp)

    po_ctr = [0]  # global PV-output slot counter (3 slots in po_bank)
    rc_pend = []  # deferred (slot, osb, mp, dma_fn): one batched reciprocal
    # per full po-bank cycle replaces three 1-element recips, whose per-
    # instruction PSUM-access charge otherwise dominates DVE time

    def flush_rc():
        if not rc_pend:
            return
        n = len(rc_pend)
        s0 = rc_pend[0][0]
        rc = rc_pool.tile([128, 3], F32, tag="rc", name="rc")
        nc.vector.reciprocal(
            rc[:, 0:n], po_banks[0][:, s0 : s0 + n, 128]
        )
        for i, (slot, osb, mp, dma_fn) in enumerate(rc_pend):
            nc.vector.tensor_scalar(
                out=osb[:, mp, :],
                in0=po_banks[0][:, slot, 0:128],
                scalar1=rc[:, i : i + 1],
                scalar2=None,
                op0=MUL,
            )
            if dma_fn is not None:
                dma_fn()
        rc_pend.clear()

    def make_pv(b, g, j, ptile, pdiag, vaug, vaugb):
        """PV for q-block j as 4 independently-schedulable pieces (one per
        128-row output subtile): fp8 DoubleRow pairs over off-diag k-tiles
        plus bf16 diagonal singles; the softmax division happens on the
        PSUM->SBUF drain (tensor_scalar); the block's output ships in one
        DMA after the last piece."""
        ppairs = (
            ptile[:, 0 : 4 * j * 512].rearrange("p (t c) -> p t c", c=512)
            if j > 0
            else None
        )
        osb_box = []

        def piece(mp):
            def emit():
                if "pv" in KSKIP:
                    return
                if mp == 0:
                    osb_box.append(
                        outsb_pool.tile([128, 4, 128], F32, tag="osb", name="osb")
                    )
                osb = osb_box[0]
                s6 = po_ctr[0] % 3
                po_ctr[0] += 1
                po_bank, slot = po_banks[0], s6
                po = po_bank[:, slot, 0:129]
                idx = 0
                for p in range(2 * j):
                    nc.tensor.matmul(
                        out=po,
                        lhsT=ppairs[:, 2 * p : 2 * p + 2, 128 * mp : 128 * mp + 128],
                        rhs=vaug[:, 2 * p : 2 * p + 2, 0:129],
                        start=(idx == 0),
                        stop=False,
                        perf_mode=DR,
                    )
                    idx += 1
                for m in range(mp + 1):
                    col = DIAG_OFF[m] + (128 * mp - 128 * m)
                    nc.tensor.matmul(
                        out=po,
                        lhsT=pdiag[:, col : col + 128],
                        rhs=vaugb[:, 4 * j + m, 0:129],
                        start=(idx == 0),
                        stop=(m == mp),
                    )
                    idx += 1
                if "outmul" in KSKIP:
                    return
                rc = rc_pool.tile([128, 1], F32, tag="rc", name="rc")
                nc.vector.reciprocal(rc, po_bank[:, slot, 128:129])
                nc.vector.tensor_scalar(
                    out=osb[:, mp, :],
                    in0=po_bank[:, slot, 0:128],
                    scalar1=rc[:, 0:1],
                    scalar2=None,
                    op0=MUL,
                )
                if mp == 3:
                    r0 = b * SEQ + 512 * j
                    nc.gpsimd.dma_start(
                        out=out[r0 : r0 + 512, g * 128 : (g + 1) * 128].rearrange(
                            "(m p) c -> p m c", p=128
                        ),
                        in_=osb,
                    )

            return emit

        return [piece(mp) for mp in range(4)]

    pending_pv = []
    PUMP_DEPTH = 3

    def pump(target, max_pops=2):
        # pops are rate-limited so PV pieces interleave with QK units: with
        # only 3 PV-output slots, a burst of 4+ consecutive pieces stalls PE
        # on the slot WAR (outmul 3 pieces back)
        pops = 0
        while len(pending_pv) > target and pops < max_pops:
            pending_pv.pop(0)()
            pops += 1

    # one-unit software pipeline over (QK, exp) units: QK of unit u+1 is
    # emitted BEFORE exp of unit u (and before any pumped PV pieces), so the
    # next scores are always in flight on PE while ACT/DVE exp the previous
    # tile, and PV pieces never sit in PE's queue ahead of the next QK.
    pending_exp = []

    def push_unit(emit_qk, emit_exp):
        emit_qk()
        pump(PUMP_DEPTH, 1)
        if pending_exp:
            pending_exp.pop(0)()
            pump(PUMP_DEPTH, 2)
        pending_exp.append(emit_exp)

    def flush_units():
        while pending_exp:
            pending_exp.pop(0)()
            pump(PUMP_DEPTH, 4)

    cur = {}
    seq_loads(0, cur, split=True)
    seq_alloc(cur)
    seq_transposes(cur, range(2))

    for b in range(B):
        ktr = cur["ktr"]
        vaug = cur["vaug"]
        vaugb = cur["vaugb"]
        qtrs = {0: cur["qtr0"]}
        qbfs = {}
        nxt = {}
        rows = slice(b * SEQ, (b + 1) * SEQ)
        for g in range(G):
            qtr = qtrs[g]
            # prefetch next head's q (or next sequence's loads); transpose
            # group 0 now, groups 1..3 interleaved into the j-loop below
            if g < G - 1:
                qbfs[g + 1] = load_cast(q[rows, (g + 1) * 128 : (g + 2) * 128])
                qtrs[g + 1] = qtr_pool.tile(
                    [128, SQ * 128], BF16, tag="qtr", name="qtrg"
                )
                build_tr(qbfs[g + 1], qtrs[g + 1], 0)
            elif b < B - 1:
                seq_loads(b + 1, nxt)
                seq_alloc(nxt)
                seq_transposes(nxt, [0])
                nxt["pending"] = True

            for bi, j in enumerate((0, 1, 2, 3)):
                ptile = (
                    pt_pool.tile([128, 12 * 512], FP8, tag="pt", name="pt")
                    if j > 0
                    else None
                )

                def mk_off(j, t0, n, ptile, ktr, qtr):
                    box = {}

                    def qk():
                        ps = box["ps"] = score_tile(n * 512)
                        for i in range(n):
                            t = t0 + i
                            nc.tensor.matmul(
                                out=ps[:, i * 512 : (i + 1) * 512],
                                lhsT=ktr[:, t * 128 : (t + 1) * 128],
                                rhs=qtr[:, 512 * j : 512 * j + 512],
                                start=True,
                                stop=True,
                            )

                    def ex():
                        if "actexp" in KSKIP:
                            return
                        nc.scalar.activation(
                            out=ptile[:, t0 * 512 : (t0 + n) * 512],
                            in_=box["ps"][:, 0 : n * 512],
                            func=EXP,
                            scale=SCALE,
                            bias=bias_ap[:, 0:1],
                        )

                    return qk, ex

                def mk_diag1(j, blockbox, ktr, qtr):
                    """Diagonal strip m=0 (512 wide) as its own 1024-unit."""
                    box = {}

                    def qk():
                        ps = box["ps"] = score_tile(512)
                        nc.tensor.matmul(
                            out=ps[:, 0:512],
                            lhsT=ktr[:, (4 * j) * 128 : (4 * j + 1) * 128],
                            rhs=qtr[:, 512 * j : 512 * j + 512],
                            start=True,
                            stop=True,
                        )

                    def ex():
                        stg = stag_pool.tile(
                            [128, 768], I32, tag="stag", name="stag"
                        )
                        if "dve1" not in KSKIP:
                         nc.vector.scalar_tensor_tensor(
                            out=stg[:, 0:512],
                            in0=box["ps"][:, 0:512],
                            scalar=A_CONST,
                            in1=bm[:, 0:512],
                            op0=MUL,
                            op1=ADD,
                        )
                        pdiag = blockbox["pdiag"] = ptd_pool.tile(
                            [128, DIAG_PACK_W], BF16, tag="ptd", name="ptd"
                        )
                        nc.gpsimd.tensor_copy(
                            out=pdiag[:, 0:512], in_=stg[:, 0:512].bitcast(F32)
                        )

                    return qk, ex

                def mk_diag2(b, g, j, bi, ptile, blockbox, ktr, qtr, vaug, vaugb):
                    """Diagonal strips m=1,3,2 packed [384|128|256] (768)."""
                    box = {}
                    segs = ((1, 0, 384), (3, 384, 128), (2, 512, 256))

                    def qk():
                        ps = box["ps"] = score_tile(768)
                        for m, off, w in segs:
                            nc.tensor.matmul(
                                out=ps[:, off : off + w],
                                lhsT=ktr[
                                    :, (4 * j + m) * 128 : (4 * j + m + 1) * 128
                                ],
                                rhs=qtr[:, 512 * j + 128 * m : 512 * j + 512],
                                start=True,
                                stop=True,
                            )

                    def ex():
                        stg = stag_pool.tile(
                            [128, 768], I32, tag="stag", name="stag"
                        )
                        nc.vector.scalar_tensor_tensor(
                            out=stg,
                            in0=box["ps"][:, 0:768],
                            scalar=A_CONST,
                            in1=bm[:, 512:1280],
                            op0=MUL,
                            op1=ADD,
                        )
                        pdiag = blockbox["pdiag"]
                        nc.gpsimd.tensor_copy(
                            out=pdiag[:, 512:1280], in_=stg.bitcast(F32)
                        )
                        # second transpose-group build for the next head /
                        # sequence (8-tile groups: only grp8=1 remains)
                        if bi == 1:
                            if g < G - 1:
                                build_tr(qbfs[g + 1], qtrs[g + 1], 1)
                            elif nxt.get("pending"):
                                seq_transposes(nxt, [1])
                        pending_pv.extend(
                            make_pv(b, g, j, ptile, pdiag, vaug, vaugb)
                        )

                    return qk, ex

                blockbox = {}
                for t0, n in off_tiling(j):
                    push_unit(*mk_off(j, t0, n, ptile, ktr, qtr))
                push_unit(*mk_diag1(j, blockbox, ktr, qtr))
                push_unit(
                    *mk_diag2(b, g, j, bi, ptile, blockbox, ktr, qtr, vaug, vaugb)
                )
        cur = nxt
    flush_units()
    pump(0, 10**9)
    flush_rc()


def build_program():
    nc = bacc.Bacc()
    q = nc.declare_dram_parameter("q", [TOK, G * HEAD_DIM], F32, isOutput=False)
    k = nc.declare_dram_parameter("k", [TOK, HEAD_DIM], F32, isOutput=False)
    v = nc.declare_dram_parameter("v", [TOK, HEAD_DIM], F32, isOutput=False)
    bmask = nc.declare_dram_parameter("bmask", [128, DIAG_PACK_W], F32, isOutput=False)
    out = nc.declare_dram_parameter("out", [TOK, G * HEAD_DIM], F32, isOutput=True)
    with tile.TileContext(nc) as tc:
        with ExitStack() as ctx:
            _body(ctx, tc, q, k, v, bmask, out)
    nc.finalize()
    return nc


_NC_CACHE = None


def _get_nc():
    global _NC_CACHE
    if _NC_CACHE is None:
        _NC_CACHE = build_program()
    return _NC_CACHE


def make_bmask():
    bm = np.full((128, DIAG_PACK_W), B_LOW, dtype=np.float32)
    kk = np.arange(128)[:, None]
    for m in range(4):
        c = np.arange(DIAG_W[m])[None, :]
        seg = np.where(c >= kk, B_CONST, B_LOW)
        bm[:, DIAG_OFF[m] : DIAG_OFF[m] + DIAG_W[m]] = seg
    return bm


def make_in_maps(q, k, v):
    q = np.ascontiguousarray(np.asarray(q, dtype=np.float32))
    k = np.ascontiguousarray(np.asarray(k, dtype=np.float32))
    v = np.ascontiguousarray(np.asarray(v, dtype=np.float32))
    bmask = make_bmask()
    in_maps = []
    for i in range(N_CORES):
        in_maps.append(
            {
                "q": np.ascontiguousarray(
                    q[:, i * G * HEAD_DIM : (i + 1) * G * HEAD_DIM]
                ),
                "k": np.ascontiguousarray(k[:, i * HEAD_DIM : (i + 1) * HEAD_DIM]),
                "v": np.ascontiguousarray(v[:, i * HEAD_DIM : (i + 1) * HEAD_DIM]),
                "bmask": bmask,
            }
        )
    return in_maps


def kernel(q, k, v, seq_len=None, **kwargs):
    res = run_bass_kernel_spmd(
        _get_nc(), make_in_maps(q, k, v), core_ids=list(range(N_CORES))
    )
    outs = [res.results[i]["out"] for i in range(N_CORES)]
    return np.concatenate(outs, axis=1)


# revision 70
# speedup vs baseline: 1.0100x; 1.0100x over previous
"""Causal GQA attention (B=2, S=2048, 32 q-heads, 8 kv-heads, D=128) on 8 TRN2 cores.

Sharding: tensor-parallel over kv heads - core i gets kv head i plus its 4
query heads (q cols [512i, 512i+512), k/v cols [128i, 128i+128)). Each core
computes its heads' attention independently; outputs concatenate on axis 1.

Per-core kernel. Differences from the 159us baseline:
  - P (softmax numerators) is fp8e4m3; PV uses fp8 DoubleRow matmuls over
    k-tile pairs (2 k-tiles per pass, 0.5 cyc/row) -> PV PE time ~4x lower.
  - exp is split across two engines: ACT does the off-diagonal strips with
    the exact table exp (fp8 out), while DVE computes diagonal strips via a
    2-instruction Schraudolph: pass1 = tensor_scalar (s*A+B) written to an
    int32 tile (the float->int convert-on-write builds the exp bit pattern;
    causal masking comes free by using scalar_tensor_tensor with a B-mask
    tile whose invalid columns hold B-50*2^23, underflowing to 0 in fp8),
    pass2 = copy of the int32 tile bitcast as f32 -> fp8 P (on Pool/gpsimd).
  - input casts (q,k->bf16, v->fp8) run on gpsimd; K^T/Q^T transpose drains
    run on gpsimd; DVE keeps pass1, PV-output normalization, reciprocals.
  - scores PSUM packing: per 512-q block j, off-diag strips fill 1536/1024
    tiles (ACT exp, one instruction per tile); the 4 diagonal suffixes pack
    into one 1536 tile as [512|384|pad128|256|128] so every matmul stays
    bank-contained and P's dense layout supports the PV pair addressing.
"""

import os
import sys

sys.path.insert(0, "/opt/trn_rl_repo")

KSKIP = set(os.environ.get("KSKIP", "").split(","))

from contextlib import ExitStack

import numpy as np

import concourse.bass as bass
import concourse.mybir as mybir
from concourse import bacc
import concourse.tile as tile
from concourse.bass_utils import run_bass_kernel_spmd
from concourse.masks import make_identity

F32 = mybir.dt.float32
BF16 = mybir.dt.bfloat16
FP8 = mybir.dt.float8e4
I32 = mybir.dt.int32

NUM_HEADS = 32
HEAD_DIM = 128
NUM_KV_HEADS = 8
SCALE = 0.08838834764831845  # 1/sqrt(128)
SEQ = 2048
TOK = 4096
B = TOK // SEQ  # 2 sequences
N_CORES = 8
G = NUM_HEADS // NUM_KV_HEADS  # 4 query heads per kv head (= per core)
SQ = SEQ // 128  # 16 128-token tiles per sequence
NQB = SEQ // 512  # 4 512-wide q blocks per sequence
EXP = mybir.ActivationFunctionType.Exp
MUL = mybir.AluOpType.mult
ADD = mybir.AluOpType.add
DR = mybir.MatmulPerfMode.DoubleRow

# Schraudolph exp constants: int32(s*A + B) bitcast to f32 ~= exp(SCALE*s).
# P carries a global 2^-3 factor (numerator and denominator both scale, the
# softmax ratio is invariant) so the largest exp stays well inside fp8e4m3.
LOG2E = 1.4426950408889634
SIGMA = 0.0434609
PSHIFT = 5.0  # fp8e4 (IEEE e4m3) max normal is 240; keep max P ~tens
A_CONST = float(SCALE * LOG2E * (1 << 23))
B_CONST = float((127.0 - SIGMA - PSHIFT) * (1 << 23))
B_LOW = float(B_CONST - 50.0 * (1 << 23))  # masked: underflows to 0 in fp8
ACT_BIAS = float(-PSHIFT * 0.6931471805599453)  # exp(SCALE*s - 3*ln2)

# diagonal pack layout (within the per-j diag PSUM tile / P region):
# gap-free order [m0:512 | m1:384 | m3:128 | m2:256] keeps every strip
# inside one 512-f32 PSUM bank
DIAG_OFF = (0, 512, 1024, 896)
DIAG_W = (512, 384, 256, 128)
DIAG_PACK_W = 1280

P_W = 12 * 512 + DIAG_PACK_W  # widest per-j P region (j=3)




def off_tiling(j):
    """Off-diagonal strip grouping into PSUM tiles: (first_tile, n_strips).
    Uniform 2-strip (1024 = 2 PSUM banks) tiles so three score regions fit
    alongside the PV-output and transpose banks."""
    return [(t, 2) for t in range(0, 4 * j, 2)]


def _body(ctx, tc, q, k, v, bmask, out):
    nc = tc.nc
    const = ctx.enter_context(tc.tile_pool(name="const", bufs=1))
    stage = ctx.enter_context(tc.tile_pool(name="stage", bufs=4))
    stagebf = ctx.enter_context(tc.tile_pool(name="stagebf", bufs=3))
    ktr_pool = ctx.enter_context(tc.tile_pool(name="ktr", bufs=2))
    qtr_pool = ctx.enter_context(tc.tile_pool(name="qtr", bufs=2))
    vaug_pool = ctx.enter_context(tc.tile_pool(name="vaug", bufs=4))
    pt_pool = ctx.enter_context(tc.tile_pool(name="pt", bufs=4))
    ptd_pool = ctx.enter_context(tc.tile_pool(name="ptd", bufs=4))
    stag_pool = ctx.enter_context(tc.tile_pool(name="stag", bufs=4))
    outsb_pool = ctx.enter_context(tc.tile_pool(name="outsb", bufs=8))
    rc_pool = ctx.enter_context(tc.tile_pool(name="rc", bufs=12))
    # PSUM: three 2-bank score regions (3-deep pipeline), 1 bank of 3
    # PV-output slots, 1 transpose staging bank.
    psum_a = ctx.enter_context(tc.tile_pool(name="psum_a", bufs=1, space="PSUM"))
    psum_b = ctx.enter_context(tc.tile_pool(name="psum_b", bufs=1, space="PSUM"))
    psum_c = ctx.enter_context(tc.tile_pool(name="psum_c", bufs=1, space="PSUM"))
    psum_o = ctx.enter_context(tc.tile_pool(name="psum_o", bufs=1, space="PSUM"))
    psum_t = ctx.enter_context(tc.tile_pool(name="psum_t", bufs=1, space="PSUM"))

    ident = const.tile([128, 128], BF16)
    make_identity(nc, ident)
    bm = const.tile([128, DIAG_PACK_W], F32)
    nc.sync.dma_start(out=bm, in_=bmask[:, :])
    bias_ap = const.tile([128, 1], F32, name="actbias")
    nc.vector.memset(bias_ap, ACT_BIAS)
    po_banks = [psum_o.tile([128, 3, 132], F32, name="po_bank0", bufs=1)]

    ab_state = [0]  # score-region rotation

    def score_tile(width):
        pool, tag = (
            (psum_a, "psa"),
            (psum_b, "psb"),
            (psum_c, "psc"),
        )[ab_state[0] % 3]
        ab_state[0] += 1
        return pool.tile([128, 1024], F32, tag=tag, name=tag)

    def build_tr(src_bf, dst_tr, grp8, drain=None):
        """Transpose 8 [128,128] bf16 tiles via one full PSUM bank, one
        drain. Few, widely-spaced builds keep the PSUM-bank WAR (which
        head-of-line blocks the in-order PE queue) off the critical path."""
        pst = psum_t.tile([128, 1024], BF16, tag="pst")
        for i in range(8):
            nc.tensor.transpose(
                out=pst[:, i * 128 : (i + 1) * 128],
                in_=src_bf[:, grp8 * 8 + i, :],
                identity=ident,
            )
        eng = drain or nc.vector  # Pool cannot read PSUM
        eng.tensor_copy(out=dst_tr[:, grp8 * 1024 : (grp8 + 1) * 1024], in_=pst)

    def load_cast(src_rows_ap, split=False):
        st = stage.tile([128, SQ, 128], F32, tag="stage")
        bf = stagebf.tile([128, SQ, 128], BF16, tag="stagebf")
        tiled = src_rows_ap.rearrange("(t p) d -> p t d", p=128)
        chunks = [(0, 4), (4, SQ)] if split else [(0, SQ)]
        for t0, t1 in chunks:
            nc.sync.dma_start(out=st[:, t0:t1, :], in_=tiled[:, t0:t1, :])
            nc.gpsimd.tensor_copy(out=bf[:, t0:t1, :], in_=st[:, t0:t1, :])
        return bf

    def seq_loads(b, dst, split=False):
        """DMA+cast K, first-head Q, and V(aug fp8) for sequence b into dst."""
        rows = slice(b * SEQ, (b + 1) * SEQ)
        if split:
            kst = stage.tile([128, SQ, 128], F32, tag="stage", name="kst")
            kbf = stagebf.tile([128, SQ, 128], BF16, tag="stagebf", name="kbf")
            qst = stage.tile([128, SQ, 128], F32, tag="stage", name="qst")
            qbf = stagebf.tile([128, SQ, 128], BF16, tag="stagebf", name="qbf")
            ktl = k[rows, :].rearrange("(t p) d -> p t d", p=128)
            qtl = q[rows, 0:128].rearrange("(t p) d -> p t d", p=128)
            for t0, t1 in ((0, 4), (4, 8), (8, 12), (12, SQ)):
                nc.sync.dma_start(out=kst[:, t0:t1, :], in_=ktl[:, t0:t1, :])
                nc.sync.dma_start(out=qst[:, t0:t1, :], in_=qtl[:, t0:t1, :])
                nc.gpsimd.tensor_copy(out=kbf[:, t0:t1, :], in_=kst[:, t0:t1, :])
                nc.gpsimd.tensor_copy(out=qbf[:, t0:t1, :], in_=qst[:, t0:t1, :])
            dst["kbf"], dst["qbf0"] = kbf, qbf
        else:
            dst["kbf"] = load_cast(k[rows, :])
            dst["qbf0"] = load_cast(q[rows, 0:128])
        vst = stage.tile([128, SQ, 128], F32, tag="stage")
        nc.sync.dma_start(out=vst, in_=v[rows, :].rearrange("(t p) d -> p t d", p=128))
        vaug = vaug_pool.tile([128, SQ, 132], FP8, name="vaug8")
        nc.gpsimd.tensor_copy(out=vaug[:, :, 0:128], in_=vst)
        nc.gpsimd.memset(vaug[:, :, 128:129], 1.0)
        dst["vaug"] = vaug
        # bf16 copy for the diagonal PV singles (fp8 V error is unaveraged on
        # the first rows of each block; bf16 V + bf16 P fixes them)
        vaugb = vaug_pool.tile([128, SQ, 132], BF16, name="vaugb")
        nc.gpsimd.tensor_copy(out=vaugb[:, :, 0:128], in_=vst)
        nc.gpsimd.memset(vaugb[:, :, 128:129], 1.0)
        dst["vaugb"] = vaugb

    def seq_alloc(dst):
        dst["ktr"] = ktr_pool.tile([128, SQ * 128], BF16, tag="ktr", name="ktr")
        dst["qtr0"] = qtr_pool.tile([128, SQ * 128], BF16, tag="qtr", name="qtr0")

    def seq_transposes(dst, grps):
        for grp in grps:
            build_tr(dst["kbf"], dst["ktr"], grp)
            build_tr(dst["qbf0"], dst["qtr0"], grp)

    po_ctr = [0]  # global PV-output slot counter (3 slots in po_bank)
    rc_pend = []  # deferred (slot, osb, mp, dma_fn): one batched reciprocal
    # per full po-bank cycle replaces three 1-element recips, whose per-
    # instruction PSUM-access charge otherwise dominates DVE time

    def flush_rc():
        if not rc_pend:
            return
        n = len(rc_pend)
        s0 = rc_pend[0][0]
        rc = rc_pool.tile([128, 3], F32, tag="rc", name="rc")
        nc.vector.reciprocal(
            rc[:, 0:n], po_banks[0][:, s0 : s0 + n, 128]
        )
        for i, (slot, osb, mp, dma_fn) in enumerate(rc_pend):
            nc.vector.tensor_scalar(
                out=osb[:, mp, :],
                in0=po_banks[0][:, slot, 0:128],
                scalar1=rc[:, i : i + 1],
                scalar2=None,
                op0=MUL,
            )
            if dma_fn is not None:
                dma_fn()
        rc_pend.clear()

    def make_pv(b, g, j, ptile, pdiag, vaug, vaugb):
        """PV for q-block j as 4 independently-schedulable pieces (one per
        128-row output subtile): fp8 DoubleRow pairs over off-diag k-tiles
        plus bf16 diagonal singles; the softmax division happens on the
        PSUM->SBUF drain (tensor_scalar); the block's output ships in one
        DMA after the last piece."""
        ppairs = (
            ptile[:, 0 : 4 * j * 512].rearrange("p (t c) -> p t c", c=512)
            if j > 0
            else None
        )
        osb_box = []

        def piece(mp):
            def emit():
                if "pv" in KSKIP:
                    return
                if mp == 0:
                    osb_box.append(
                        outsb_pool.tile([128, 4, 128], F32, tag="osb", name="osb")
                    )
                osb = osb_box[0]
                s6 = po_ctr[0] % 3
                po_ctr[0] += 1
                po_bank, slot = po_banks[0], s6
                po = po_bank[:, slot, 0:129]
                idx = 0
                for p in range(2 * j):
                    nc.tensor.matmul(
                        out=po,
                        lhsT=ppairs[:, 2 * p : 2 * p + 2, 128 * mp : 128 * mp + 128],
                        rhs=vaug[:, 2 * p : 2 * p + 2, 0:129],
                        start=(idx == 0),
                        stop=False,
                        perf_mode=DR,
                    )
                    idx += 1
                for m in range(mp + 1):
                    col = DIAG_OFF[m] + (128 * mp - 128 * m)
                    nc.tensor.matmul(
                        out=po,
                        lhsT=pdiag[:, col : col + 128],
                        rhs=vaugb[:, 4 * j + m, 0:129],
                        start=(idx == 0),
                        stop=(m == mp),
                    )
                    idx += 1
                if "outmul" in KSKIP:
                    return
                rc = rc_pool.tile([128, 1], F32, tag="rc", name="rc")
                nc.vector.reciprocal(rc, po_bank[:, slot, 128:129])
                nc.vector.tensor_scalar(
                    out=osb[:, mp, :],
                    in0=po_bank[:, slot, 0:128],
                    scalar1=rc[:, 0:1],
                    scalar2=None,
                    op0=MUL,
                )
                if mp == 3:
                    r0 = b * SEQ + 512 * j
                    nc.gpsimd.dma_start(
                        out=out[r0 : r0 + 512, g * 128 : (g + 1) * 128].rearrange(
                            "(m p) c -> p m c", p=128
                        ),
                        in_=osb,
                    )

            return emit

        return [piece(mp) for mp in range(4)]

    pending_pv = []
    PUMP_DEPTH = 3

    def pump(target, max_pops=2):
        # pops are rate-limited so PV pieces interleave with QK units: with
        # only 3 PV-output slots, a burst of 4+ consecutive pieces stalls PE
        # on the slot WAR (outmul 3 pieces back)
        pops = 0
        while len(pending_pv) > target and pops < max_pops:
            pending_pv.pop(0)()
            pops += 1

    # one-unit software pipeline over (QK, exp) units: QK of unit u+1 is
    # emitted BEFORE exp of unit u (and before any pumped PV pieces), so the
    # next scores are always in flight on PE while ACT/DVE exp the previous
    # tile, and PV pieces never sit in PE's queue ahead of the next QK.
    pending_exp = []

    def push_unit(emit_qk, emit_exp):
        emit_qk()
        pump(PUMP_DEPTH, 1)
        if pending_exp:
            pending_exp.pop(0)()
            pump(PUMP_DEPTH, 2)
        pending_exp.append(emit_exp)

    def flush_units():
        while pending_exp:
            pending_exp.pop(0)()
            pump(PUMP_DEPTH, 4)

    cur = {}
    seq_loads(0, cur, split=True)
    seq_alloc(cur)
    # cold start: half-size (4-tile) transpose builds so the first q-block's
    # QK (which needs only tiles 0-3) is not gated on full 8-tile groups
    for grp4 in (0, 1):
        for src_bf, dst_tr in ((cur["kbf"], cur["ktr"]), (cur["qbf0"], cur["qtr0"])):
            pst4 = psum_t.tile([128, 1024], BF16, tag="pst", name="pst4")
            for i in range(4):
                nc.tensor.transpose(
                    out=pst4[:, i * 128 : (i + 1) * 128],
                    in_=src_bf[:, grp4 * 4 + i, :],
                    identity=ident,
                )
            nc.vector.tensor_copy(
                out=dst_tr[:, grp4 * 512 : (grp4 + 1) * 512], in_=pst4[:, 0:512]
            )
    seq_transposes(cur, [1])

    for b in range(B):
        ktr = cur["ktr"]
        vaug = cur["vaug"]
        vaugb = cur["vaugb"]
        qtrs = {0: cur["qtr0"]}
        qbfs = {}
        nxt = {}
        rows = slice(b * SEQ, (b + 1) * SEQ)
        for g in range(G):
            qtr = qtrs[g]
            # prefetch next head's q (or next sequence's loads); transpose
            # group 0 now, groups 1..3 interleaved into the j-loop below
            if g < G - 1:
                qbfs[g + 1] = load_cast(q[rows, (g + 1) * 128 : (g + 2) * 128])
                qtrs[g + 1] = qtr_pool.tile(
                    [128, SQ * 128], BF16, tag="qtr", name="qtrg"
                )
                build_tr(qbfs[g + 1], qtrs[g + 1], 0)
            elif b < B - 1:
                seq_loads(b + 1, nxt)
                seq_alloc(nxt)
                seq_transposes(nxt, [0])
                nxt["pending"] = True

            for bi, j in enumerate((0, 1, 2, 3)):
                ptile = (
                    pt_pool.tile([128, 12 * 512], FP8, tag="pt", name="pt")
                    if j > 0
                    else None
                )

                def mk_off(j, t0, n, ptile, ktr, qtr):
                    box = {}

                    def qk():
                        ps = box["ps"] = score_tile(n * 512)
                        for i in range(n):
                            t = t0 + i
                            nc.tensor.matmul(
                                out=ps[:, i * 512 : (i + 1) * 512],
                                lhsT=ktr[:, t * 128 : (t + 1) * 128],
                                rhs=qtr[:, 512 * j : 512 * j + 512],
                                start=True,
                                stop=True,
                            )

                    def ex():
                        if "actexp" in KSKIP:
                            return
                        nc.scalar.activation(
                            out=ptile[:, t0 * 512 : (t0 + n) * 512],
                            in_=box["ps"][:, 0 : n * 512],
                            func=EXP,
                            scale=SCALE,
                            bias=bias_ap[:, 0:1],
                        )

                    return qk, ex

                def mk_diag1(j, blockbox, ktr, qtr):
                    """Diagonal strip m=0 (512 wide) as its own 1024-unit."""
                    box = {}

                    def qk():
                        ps = box["ps"] = score_tile(512)
                        nc.tensor.matmul(
                            out=ps[:, 0:512],
                            lhsT=ktr[:, (4 * j) * 128 : (4 * j + 1) * 128],
                            rhs=qtr[:, 512 * j : 512 * j + 512],
                            start=True,
                            stop=True,
                        )

                    def ex():
                        stg = stag_pool.tile(
                            [128, 768], I32, tag="stag", name="stag"
                        )
                        if "dve1" not in KSKIP:
                         nc.vector.scalar_tensor_tensor(
                            out=stg[:, 0:512],
                            in0=box["ps"][:, 0:512],
                            scalar=A_CONST,
                            in1=bm[:, 0:512],
                            op0=MUL,
                            op1=ADD,
                        )
                        pdiag = blockbox["pdiag"] = ptd_pool.tile(
                            [128, DIAG_PACK_W], BF16, tag="ptd", name="ptd"
                        )
                        nc.gpsimd.tensor_copy(
                            out=pdiag[:, 0:512], in_=stg[:, 0:512].bitcast(F32)
                        )

                    return qk, ex

                def mk_diag2(b, g, j, bi, ptile, blockbox, ktr, qtr, vaug, vaugb):
                    """Diagonal strips m=1,3,2 packed [384|128|256] (768)."""
                    box = {}
                    segs = ((1, 0, 384), (3, 384, 128), (2, 512, 256))

                    def qk():
                        ps = box["ps"] = score_tile(768)
                        for m, off, w in segs:
                            nc.tensor.matmul(
                                out=ps[:, off : off + w],
                                lhsT=ktr[
                                    :, (4 * j + m) * 128 : (4 * j + m + 1) * 128
                                ],
                                rhs=qtr[:, 512 * j + 128 * m : 512 * j + 512],
                                start=True,
                                stop=True,
                            )

                    def ex():
                        stg = stag_pool.tile(
                            [128, 768], I32, tag="stag", name="stag"
                        )
                        nc.vector.scalar_tensor_tensor(
                            out=stg,
                            in0=box["ps"][:, 0:768],
                            scalar=A_CONST,
                            in1=bm[:, 512:1280],
                            op0=MUL,
                            op1=ADD,
                        )
                        pdiag = blockbox["pdiag"]
                        nc.gpsimd.tensor_copy(
                            out=pdiag[:, 512:1280], in_=stg.bitcast(F32)
                        )
                        # second transpose-group build for the next head /
                        # sequence (8-tile groups: only grp8=1 remains)
                        if bi == 1:
                            if g < G - 1:
                                build_tr(qbfs[g + 1], qtrs[g + 1], 1)
                            elif nxt.get("pending"):
                                seq_transposes(nxt, [1])
                        pending_pv.extend(
                            make_pv(b, g, j, ptile, pdiag, vaug, vaugb)
                        )

                    return qk, ex

                blockbox = {}
                for t0, n in off_tiling(j):
                    push_unit(*mk_off(j, t0, n, ptile, ktr, qtr))
                push_unit(*mk_diag1(j, blockbox, ktr, qtr))
                push_unit(
                    *mk_diag2(b, g, j, bi, ptile, blockbox, ktr, qtr, vaug, vaugb)
                )
        cur = nxt
    flush_units()
    pump(0, 10**9)
    flush_rc()


def build_program():
    nc = bacc.Bacc()
    q = nc.declare_dram_parameter("q", [TOK, G * HEAD_DIM], F32, isOutput=False)
    k = nc.declare_dram_parameter("k", [TOK, HEAD_DIM], F32, isOutput=False)
    v = nc.declare_dram_parameter("v", [TOK, HEAD_DIM], F32, isOutput=False)
    bmask = nc.declare_dram_parameter("bmask", [128, DIAG_PACK_W], F32, isOutput=False)
    out = nc.declare_dram_parameter("out", [TOK, G * HEAD_DIM], F32, isOutput=True)
    with tile.TileContext(nc) as tc:
        with ExitStack() as ctx:
            _body(ctx, tc, q, k, v, bmask, out)
    nc.finalize()
    return nc


_NC_CACHE = None


def _get_nc():
    global _NC_CACHE
    if _NC_CACHE is None:
        _NC_CACHE = build_program()
    return _NC_CACHE


def make_bmask():
    bm = np.full((128, DIAG_PACK_W), B_LOW, dtype=np.float32)
    kk = np.arange(128)[:, None]
    for m in range(4):
        c = np.arange(DIAG_W[m])[None, :]
        seg = np.where(c >= kk, B_CONST, B_LOW)
        bm[:, DIAG_OFF[m] : DIAG_OFF[m] + DIAG_W[m]] = seg
    return bm


def make_in_maps(q, k, v):
    q = np.ascontiguousarray(np.asarray(q, dtype=np.float32))
    k = np.ascontiguousarray(np.asarray(k, dtype=np.float32))
    v = np.ascontiguousarray(np.asarray(v, dtype=np.float32))
    bmask = make_bmask()
    in_maps = []
    for i in range(N_CORES):
        in_maps.append(
            {
                "q": np.ascontiguousarray(
                    q[:, i * G * HEAD_DIM : (i + 1) * G * HEAD_DIM]
                ),
                "k": np.ascontiguousarray(k[:, i * HEAD_DIM : (i + 1) * HEAD_DIM]),
                "v": np.ascontiguousarray(v[:, i * HEAD_DIM : (i + 1) * HEAD_DIM]),
                "bmask": bmask,
            }
        )
    return in_maps


def kernel(q, k, v, seq_len=None, **kwargs):
    res = run_bass_kernel_spmd(
        _get_nc(), make_in_maps(q, k, v), core_ids=list(range(N_CORES))
    )
    outs = [res.results[i]["out"] for i in range(N_CORES)]
    return np.concatenate(outs, axis=1)
